# revision 21
# baseline (speedup 1.0000x reference)
"""LocalAttention2d Bass kernel for 8 Trainium2 NeuronCores.

Strategy: pure data parallel over batch (8 batches/core).  The module only
attends over an 8x8 window of data-dependent spatial positions per batch.
All valid window columns are literally p1+offs (clipping only produces
invalid, masked positions), so each (batch, window-row, col-half) is ONE
contiguous 4-row block of the spatial table.  The kernel computes 128 block
indices on-device and fetches all 512 needed rows with a SINGLE indirect
DMA of 128 x 2KB descriptors from a host-padded [B*(H*W+16), D] table (the
+-8 row pads absorb edge blocks; those positions are invalid and masked).

Layout: partition p = b*16 + i*2 + jg (batch, window-row, col-half); the
index math runs on tiny [16, 8] r/c tiles, is expanded to [128, 1] block
indices by one selector matmul, and the Gaussian shift + validity mask are
folded into the scores PSUM with per-batch-constant and banded matmuls
using the linear mask -1024 + 512*(vr+vc) (exact in f32 at this scale), so
exp() reads PSUM directly.  Softmax is unnormalized: the weighted sum and
the denominator are both matmuls against exp scores, normalized at the end
by a per-partition reciprocal.  Score and weighted-sum matmuls run in
fp32r (single PE pass).

Host-side work is limited to data-INdependent layout prep (transposes of
q / c_t / W_p, constant tables, zero padding); every data-dependent step
(p_t, rounding, window indices, shift, softmax, output) runs on-device.
"""

import sys

import numpy as np

try:
    import concourse.bass_utils as _bu
except ImportError:  # fresh grading dir: fall back to the repo checkout
    sys.path.insert(0, "/opt/trn_rl_repo")
    import concourse.bass_utils as _bu

import concourse.bacc as bacc
import concourse.bass as bass
import concourse.mybir as mybir
import concourse.tile as tile
from concourse.bass import IndirectOffsetOnAxis

B, D, H, W = 64, 128, 128, 128
CSZ = 256
R = 8                     # window rows == cols
NCORES = 8
BPC = B // NCORES         # batches per core
HW = H * W
PAD = 8                   # zero rows before/after each batch in the table
BSTR = HW + 2 * PAD       # padded batch stride (rows)
ROWS = H + 1              # 129, padded row count in the reference
NS = 4                    # strips (col within 4-row block)
F32 = mybir.dt.float32
F32R = mybir.dt.float32r
I32 = mybir.dt.int32

AOP = mybir.AluOpType
ACT = mybir.ActivationFunctionType
AXL = mybir.AxisListType

BIGC = 1024.0             # mask constants: exact cancellation at 2^10 scale
HALFC = 512.0
FAKEC = -2048.0

# auxS [128, 174] (critical-path constants, first DMA):
#   0:8 ct0 | 8:16 ct1 | 16:18 wp0 | 18:20 wp1 | 20:36 selc16 (parts 0:2)
#   | 36:37 oi16 (parts 0:16) | 37:165 E1 | 165:293 E2 (parts 0:16)
#   | 293:301 selmask | 301:302 jgboff
AUXS_W = 302
# auxL [128, 1064]: 0:128 wa0 | 128:256 wa1 | 256:384 ident
#   | 384:896 B_s x4 (parts 0:16) | 896:1024 bsel (parts 0:8)
#   | 1024:1056 constm8 (parts 0:8) | 1056:1064 fold8 (parts 0:32)
AUXL_W = 1064


def _build():
    nc = bacc.Bacc(
        "TRN2",
        target_bir_lowering=False,
        debug=False,
        num_devices=NCORES,
    )

    qtab = nc.dram_tensor("qtab", [BPC * BSTR, D], F32, kind="ExternalInput")
    auxS = nc.dram_tensor("auxS", [128, AUXS_W], F32, kind="ExternalInput")
    auxL = nc.dram_tensor("auxL", [128, AUXL_W], F32, kind="ExternalInput")
    out = nc.dram_tensor("out", [BPC, D], F32, kind="ExternalOutput")

    with tile.TileContext(nc) as tc:
        with (
            tc.tile_pool(name="sb", bufs=1) as sp,
            tc.tile_pool(name="ps", bufs=1, space="PSUM") as pp,
        ):
            # ---- input DMAs: small critical constants first ---------------
            aS = sp.tile([128, AUXS_W], F32)
            nc.sync.dma_start(out=aS[:], in_=auxS[:])
            aL = sp.tile([128, AUXL_W], F32)
            nc.sync.dma_start(out=aL[:], in_=auxL[:])

            ct0 = aS[:, 0:8]
            ct1 = aS[:, 8:16]
            wp0 = aS[:, 16:18]
            wp1 = aS[:, 18:20]
            selc16 = aS[0:2, 20:36]
            oi16 = aS[0:16, 36:37]
            E1 = aS[0:16, 37:165]
            E2 = aS[0:16, 165:293]
            selmask = aS[:, 293:301]
            jgboff = aS[:, 301:302]
            wa0 = aL[:, 0:128]
            wa1 = aL[:, 128:256]
            ident = aL[:, 256:384]
            bsel = aL[0:8, 896:1024]
            constm8 = aL[0:8, 1024:1056]
            fold8 = aL[0:32, 1056:1064]

            # ---- prefetch the Exp activation table (scalar engine would
            # otherwise load it mid-kernel, stalling the sigmoid handoff) ---
            dmy = sp.tile([2, 2], F32)
            nc.gpsimd.memset(dmy[:], 0.0)
            dmye = sp.tile([2, 2], F32)
            nc.scalar.activation(out=dmye[:], in_=dmy[:], func=ACT.Exp)

            ones2 = sp.tile([128, 2], F32R)
            nc.gpsimd.memset(ones2[:].bitcast(F32), 1.0)

            # ---- p_t: ptT[2,8] = (W_p c_t)^T, spread 128*sigmoid to [16,8] -
            ptcomb = pp.tile([16, 2 * BPC], F32)
            ptT_ps = ptcomb[0:2, BPC : 2 * BPC]
            nc.tensor.matmul(out=ptT_ps, lhsT=wp0, rhs=ct0, start=True, stop=False, skip_group_check=True)
            nc.tensor.matmul(out=ptT_ps, lhsT=wp1, rhs=ct1, start=False, stop=True, skip_group_check=True)
            sig8 = sp.tile([2, BPC], F32)
            nc.scalar.activation(out=sig8[:], in_=ptT_ps, func=ACT.Sigmoid)
            # rows 0:8 = r-part (p_t[:,0]), rows 8:16 = c-part (p_t[:,1])
            ptb_ps = ptcomb[0:16, 0:BPC]
            nc.tensor.matmul(out=ptb_ps, lhsT=selc16, rhs=sig8[:], start=True, stop=True, skip_group_check=True)

            # ---- round via the +-2^23 trick (one op) ----------------------
            prf = sp.tile([16, BPC], F32)
            nc.vector.tensor_scalar(
                out=prf[:], in0=ptb_ps, scalar1=8388608.0, scalar2=8388608.0,
                op0=AOP.add, op1=AOP.subtract,
            )

            # ---- window values: a=max(p+o,0); rr=a*(a<129); rm1=max(rr-1,0)
            aa = sp.tile([16, BPC], F32)
            nc.vector.tensor_scalar(
                out=aa[:], in0=prf[:], scalar1=oi16, scalar2=0.0,
                op0=AOP.add, op1=AOP.max,
            )
            amask = sp.tile([16, BPC], F32)
            nc.vector.tensor_scalar(
                out=amask[:], in0=aa[:], scalar1=float(ROWS), scalar2=None, op0=AOP.is_lt
            )
            rr = sp.tile([16, BPC], F32)
            nc.vector.tensor_tensor(out=rr[:], in0=aa[:], in1=amask[:], op=AOP.mult)
            rmX = sp.tile([16, BPC], F32)
            nc.vector.tensor_scalar(
                out=rmX[:], in0=rr[:], scalar1=1.0, scalar2=0.0,
                op0=AOP.subtract, op1=AOP.max,
            )

            # ---- block indices: two selector matmuls + diag pick ----------
            # idx8[p, b'] = 128*rm1_r[i(p), b'] + p1[b']
            comb2 = pp.tile([128, BPC + 2], F32)
            idx8_ps = comb2[:, 0:BPC]
            nc.tensor.matmul(out=idx8_ps, lhsT=E1, rhs=rmX[:], start=True, stop=False, skip_group_check=True)
            nc.tensor.matmul(out=idx8_ps, lhsT=E2, rhs=prf[:], start=False, stop=True, skip_group_check=True)
            m1 = sp.tile([128, BPC], F32)
            nc.vector.tensor_tensor(out=m1[:], in0=idx8_ps, in1=selmask, op=AOP.mult)
            red = sp.tile([128, 1], F32)
            nc.vector.tensor_reduce(out=red[:], in_=m1[:], axis=AXL.X, op=AOP.add)
            idxs = sp.tile([128, 1], F32)
            nc.vector.tensor_scalar(
                out=idxs[:], in0=red[:], scalar1=jgboff, scalar2=None, op0=AOP.add
            )
            idx128 = sp.tile([128, 1], I32)
            nc.vector.tensor_copy(idx128[:], idxs[:])

            # ---- THE gather: one DMA, 128 x 2KB blocks --------------------
            qgB = sp.tile([128, NS * D], F32R)
            nc.gpsimd.indirect_dma_start(
                out=qgB[:],
                out_offset=None,
                in_=qtab[:].bitcast(F32R),
                in_offset=IndirectOffsetOnAxis(ap=idx128[:, 0:1], axis=0),
            )

            # ---- shift/valid pre-term (overlaps the gather) ---------------
            # pre16 = 512*(rr>0) - (rm1 - p_t)^2/8 per r/c component
            rpos = sp.tile([16, BPC], F32)
            nc.vector.tensor_scalar(
                out=rpos[:], in0=rr[:], scalar1=0.0, scalar2=None, op0=AOP.is_gt
            )
            rexpd = sp.tile([16, BPC], F32)
            nc.vector.tensor_tensor(out=rexpd[:], in0=rmX[:], in1=ptb_ps, op=AOP.subtract)
            sq = sp.tile([16, BPC], F32)
            nc.vector.tensor_tensor(out=sq[:], in0=rexpd[:], in1=rexpd[:], op=AOP.mult)
            tsA = sp.tile([16, BPC], F32)
            nc.vector.tensor_scalar(
                out=tsA[:], in0=sq[:], scalar1=-0.125, scalar2=None, op0=AOP.mult
            )
            pre16 = sp.tile([16, BPC], F32)
            nc.vector.scalar_tensor_tensor(
                out=pre16[:], in0=rpos[:], scalar=HALFC, in1=tsA[:],
                op0=AOP.mult, op1=AOP.add,
            )

            # ---- vT[d,b] = sum_c W_a[c,d] c_t[c,b] ------------------------
            vT_ps = pp.tile([D, BPC], F32)
            nc.tensor.matmul(out=vT_ps[:], lhsT=wa0, rhs=ct0, start=True, stop=False)
            nc.tensor.matmul(out=vT_ps[:], lhsT=wa1, rhs=ct1, start=False, stop=True)
            vT_sb = sp.tile([D, BPC], F32R)
            nc.vector.tensor_copy(vT_sb[:], vT_ps[:])

            # ---- scores PSUM: per-batch consts + banded shift/mask --------
            # (1024-scale constants: order-independent, no cancellation loss)
            scores_ps = pp.tile([128, NS * BPC], F32)
            nc.tensor.matmul(
                out=scores_ps[:], lhsT=bsel, rhs=constm8,
                start=True, stop=False, skip_group_check=True,
            )
            for s in range(NS):
                nc.tensor.matmul(
                    out=scores_ps[:, s * BPC : (s + 1) * BPC],
                    lhsT=aL[0:16, 384 + 128 * s : 384 + 128 * (s + 1)],
                    rhs=pre16[:],
                    start=False, stop=False, skip_group_check=True,
                )

            # ---- transpose gathered strips, score matmuls (fp32r) ---------
            qgT_sb = sp.tile([D, NS * 128], F32R)
            for s in range(NS):
                tr_ps = pp.tile([D, 128], F32, tag=f"tr{s % 2}")
                nc.tensor.transpose(tr_ps[:], qgB[:, s * D : (s + 1) * D].bitcast(F32), ident)
                nc.vector.tensor_copy(qgT_sb[:, s * 128 : (s + 1) * 128], tr_ps[:])
                nc.tensor.matmul(
                    out=scores_ps[:, s * BPC : (s + 1) * BPC],
                    lhsT=qgT_sb[:, s * 128 : (s + 1) * 128],
                    rhs=vT_sb[:],
                    start=False, stop=(s == NS - 1), skip_group_check=True,
                )

            # ---- exp straight out of PSUM ---------------------------------
            e_sb = sp.tile([128, NS * BPC], F32R)
            nc.scalar.activation(out=e_sb[:], in_=scores_ps[:], func=ACT.Exp)

            # ---- denominator: S32[8s+b'] = sum_p e[p, 8s+b'] --------------
            S32_ps = comb2[0 : NS * BPC, BPC : BPC + 2]
            nc.tensor.matmul(out=S32_ps, lhsT=e_sb[:], rhs=ones2[:], start=True, stop=True, skip_group_check=True)

            # ---- unnormalized weighted sum (fp32r) ------------------------
            out_ps = pp.tile([BPC, D], F32)
            for s in range(NS):
                nc.tensor.matmul(
                    out=out_ps[:],
                    lhsT=e_sb[:, s * BPC : (s + 1) * BPC],
                    rhs=qgB[:, s * D : (s + 1) * D],
                    start=(s == 0), stop=(s == NS - 1),
                )

            # ---- fold strip sums via a constant matmul, normalize, store --
            S32_sb = sp.tile([NS * BPC, 2], F32)
            nc.vector.tensor_copy(S32_sb[:], S32_ps)
            S8_ps = pp.tile([BPC, 2], F32)
            nc.tensor.matmul(out=S8_ps[:], lhsT=fold8, rhs=S32_sb[:], start=True, stop=True)
            sinv = sp.tile([BPC, 1], F32)
            nc.vector.reciprocal(sinv[:], S8_ps[:, 0:1])
            outf = sp.tile([BPC, D], F32)
            nc.vector.tensor_scalar(
                out=outf[:], in0=out_ps[:], scalar1=sinv[:, 0:1], scalar2=None,
                op0=AOP.mult,
            )
            nc.sync.dma_start(out=out[:], in_=outf[:])

    nc.compile()
    return nc


_CACHE = {}


def _prep_in_maps(q, c_t, W_a, W_p):
    offs = (np.arange(R) - (R // 2 - 1)).astype(np.float32)  # [-3..4]
    p = np.arange(128)
    b_of = p // 16
    i_of = (p % 16) // 2
    jg_of = p % 2

    selc16_np = np.zeros((2, 16), np.float32)
    selc16_np[0, 0:8] = float(H)
    selc16_np[1, 8:16] = float(H)

    oi16_np = offs[np.arange(16) % 8]

    E1_np = np.zeros((16, 128), np.float32)
    E1_np[i_of, p] = float(W)       # 128 * rm1_r[i(p), :]
    E2_np = np.zeros((16, 128), np.float32)
    E2_np[8, :] = 1.0               # + p1 (prf row 8 = rounded c-center)

    selmask_np = (np.arange(BPC)[None, :] == b_of[:, None]).astype(np.float32)
    jgboff_np = (b_of * BSTR + PAD - 4 + 4 * jg_of).astype(np.float32)

    auxS_np = np.zeros((128, AUXS_W), np.float32)
    auxS_np[:, 16:18] = W_p.T.astype(np.float32)[0:128]
    auxS_np[:, 18:20] = W_p.T.astype(np.float32)[128:256]
    auxS_np[0:2, 20:36] = selc16_np
    auxS_np[0:16, 36] = oi16_np
    auxS_np[0:16, 37:165] = E1_np
    auxS_np[0:16, 165:293] = E2_np
    auxS_np[:, 293:301] = selmask_np
    auxS_np[:, 301] = jgboff_np

    # B_s[q, p] = d(q, i(p)) + d(q, 8 + j(p, s)),  j = 4*jg(p) + s
    auxL_np = np.zeros((128, AUXL_W), np.float32)
    auxL_np[:, 0:128] = W_a.astype(np.float32)[0:128]
    auxL_np[:, 128:256] = W_a.astype(np.float32)[128:256]
    auxL_np[:, 256:384] = np.eye(128, dtype=np.float32)
    for s in range(NS):
        Bs = np.zeros((16, 128), np.float32)
        Bs[i_of, p] += 1.0
        Bs[8 + 4 * jg_of + s, p] += 1.0
        auxL_np[0:16, 384 + 128 * s : 384 + 128 * (s + 1)] = Bs
    bsel_np = (b_of[None, :] == np.arange(8)[:, None]).astype(np.float32)
    auxL_np[0:8, 896:1024] = bsel_np
    constm8_np = np.full((8, NS * BPC), FAKEC, np.float32)
    for r in range(8):
        for s in range(NS):
            constm8_np[r, 8 * s + r] = -BIGC
    auxL_np[0:8, 1024:1056] = constm8_np
    fold8_np = np.zeros((32, 8), np.float32)
    fold8_np[np.arange(32), np.arange(32) % 8] = 1.0
    auxL_np[0:32, 1056:1064] = fold8_np

    in_maps = []
    for c in range(NCORES):
        qs = q[c * BPC : (c + 1) * BPC]  # [BPC, D, H, W]
        qhw_np = np.ascontiguousarray(qs.transpose(0, 2, 3, 1)).reshape(BPC, HW, D)
        qtab_np = np.zeros((BPC, BSTR, D), np.float32)
        qtab_np[:, PAD : PAD + HW, :] = qhw_np
        ctT_np = np.ascontiguousarray(c_t[c * BPC : (c + 1) * BPC].T)  # [CSZ, BPC]
        auxS_c = auxS_np.copy()
        auxS_c[:, 0:8] = ctT_np[0:128]
        auxS_c[:, 8:16] = ctT_np[128:256]
        in_maps.append({
            "qtab": qtab_np.reshape(BPC * BSTR, D),
            "auxS": auxS_c,
            "auxL": auxL_np,
        })
    return in_maps


def run(trace=False, tmpdir=None, **inputs):
    q = np.asarray(inputs["q"], dtype=np.float32)
    c_t = np.asarray(inputs["c_t"], dtype=np.float32)
    W_a = np.asarray(inputs["W_a"], dtype=np.float32)
    W_p = np.asarray(inputs["W_p"], dtype=np.float32)
    if "nc" not in _CACHE:
        _CACHE["nc"] = _build()
    in_maps = _prep_in_maps(q, c_t, W_a, W_p)
    res = _bu.run_bass_kernel_spmd(
        _CACHE["nc"], in_maps, core_ids=list(range(NCORES)), trace=trace,
        tmpdir=tmpdir,
    )
    outp = np.concatenate([r["out"] for r in res.results], axis=0)
    return outp, res


def kernel(**inputs):
    outp, _ = run(trace=False, **inputs)
    return outp


# revision 22
# speedup vs baseline: 1.1692x; 1.1692x over previous
"""LocalAttention2d Bass kernel for 8 Trainium2 NeuronCores.

Strategy: pure data parallel over batch (8 batches/core).  The module only
attends over an 8x8 window of data-dependent spatial positions per batch.
All valid window columns are literally p1+offs (clipping only produces
invalid, masked positions), so each (batch, window-row, col-half) is ONE
contiguous 4-row block of the spatial table.  The kernel computes 128 block
indices on-device and fetches all 512 needed rows with a SINGLE indirect
DMA of 128 x 2KB descriptors from a host-padded [B*(H*W+16), D] table (the
+-8 row pads absorb edge blocks; those positions are invalid and masked).

Layout: partition p = b*16 + i*2 + jg (batch, window-row, col-half); the
index math runs on tiny [16, 8] r/c tiles, is expanded to [128, 1] block
indices by one selector matmul, and the Gaussian shift + validity mask are
folded into the scores PSUM with per-batch-constant and banded matmuls
using the linear mask -1024 + 512*(vr+vc) (exact in f32 at this scale), so
exp() reads PSUM directly.  Softmax is unnormalized: the weighted sum and
the denominator are both matmuls against exp scores, normalized at the end
by a per-partition reciprocal.  Score and weighted-sum matmuls run in
fp32r (single PE pass).

Host-side work is limited to data-INdependent layout prep (transposes of
q / c_t / W_p, constant tables, zero padding); every data-dependent step
(p_t, rounding, window indices, shift, softmax, output) runs on-device.
"""

import sys

import numpy as np

try:
    import concourse.bass_utils as _bu
except ImportError:  # fresh grading dir: fall back to the repo checkout
    sys.path.insert(0, "/opt/trn_rl_repo")
    import concourse.bass_utils as _bu

import concourse.bacc as bacc
import concourse.bass as bass
import concourse.mybir as mybir
import concourse.tile as tile
from concourse.bass import IndirectOffsetOnAxis

B, D, H, W = 64, 128, 128, 128
CSZ = 256
R = 8                     # window rows == cols
NCORES = 8
BPC = B // NCORES         # batches per core
HW = H * W
PAD = 8                   # zero rows before/after each batch in the table
BSTR = HW + 2 * PAD       # padded batch stride (rows)
ROWS = H + 1              # 129, padded row count in the reference
NS = 4                    # strips (col within 4-row block)
F32 = mybir.dt.float32
F32R = mybir.dt.float32r
I32 = mybir.dt.int32

AOP = mybir.AluOpType
ACT = mybir.ActivationFunctionType
AXL = mybir.AxisListType

BIGC = 1024.0             # mask constants: exact cancellation at 2^10 scale
HALFC = 512.0
FAKEC = -2048.0

# auxS [128, 174] (critical-path constants, first DMA):
#   0:8 ct0 | 8:16 ct1 | 16:18 wp0 | 18:20 wp1 | 20:36 selc16 (parts 0:2)
#   | 36:37 oi16 (parts 0:16) | 37:165 E12c (parts 0:16)
#   | 165:173 selmask | 173:174 jgboff
AUXS_W = 174
# auxL [128, 1064]: 0:128 wa0 | 128:256 wa1 | 256:384 ident
#   | 384:896 B_s x4 (parts 0:16) | 896:1024 bsel (parts 0:8)
#   | 1024:1056 constm8 (parts 0:8) | 1056:1064 fold8 (parts 0:32)
AUXL_W = 1064


def _build():
    nc = bacc.Bacc(
        "TRN2",
        target_bir_lowering=False,
        debug=False,
        num_devices=NCORES,
    )

    qtab = nc.dram_tensor("qtab", [BPC * BSTR, D], F32, kind="ExternalInput")
    auxS = nc.dram_tensor("auxS", [128, AUXS_W], F32, kind="ExternalInput")
    auxL = nc.dram_tensor("auxL", [128, AUXL_W], F32, kind="ExternalInput")
    out = nc.dram_tensor("out", [BPC, D], F32, kind="ExternalOutput")

    with tile.TileContext(nc) as tc:
        with (
            tc.tile_pool(name="sb", bufs=1) as sp,
            tc.tile_pool(name="ps", bufs=1, space="PSUM") as pp,
        ):
            # ---- input DMAs: small critical constants first ---------------
            aS = sp.tile([128, AUXS_W], F32)
            nc.sync.dma_start(out=aS[:], in_=auxS[:])
            aL = sp.tile([128, AUXL_W], F32)
            nc.sync.dma_start(out=aL[:], in_=auxL[:])

            ct0 = aS[:, 0:8]
            ct1 = aS[:, 8:16]
            wp0 = aS[:, 16:18]
            wp1 = aS[:, 18:20]
            selc16 = aS[0:2, 20:36]
            oi16 = aS[0:16, 36:37]
            E12c = aS[0:16, 37:165]
            selmask = aS[:, 165:173]
            jgboff = aS[:, 173:174]
            wa0 = aL[:, 0:128]
            wa1 = aL[:, 128:256]
            ident = aL[:, 256:384]
            bsel = aL[0:8, 896:1024]
            constm8 = aL[0:8, 1024:1056]
            fold8 = aL[0:32, 1056:1064]

            ones2 = sp.tile([128, 2], F32R)
            nc.gpsimd.memset(ones2[:].bitcast(F32), 1.0)

            # ---- p_t: ptT[2,8] = (W_p c_t)^T, spread 128*sigmoid to [16,8] -
            ptcomb = pp.tile([16, 2 * BPC], F32)
            ptT_ps = ptcomb[0:2, BPC : 2 * BPC]
            nc.tensor.matmul(out=ptT_ps, lhsT=wp0, rhs=ct0, start=True, stop=False, skip_group_check=True)
            nc.tensor.matmul(out=ptT_ps, lhsT=wp1, rhs=ct1, start=False, stop=True, skip_group_check=True)
            sig8 = sp.tile([2, BPC], F32)
            nc.scalar.activation(out=sig8[:], in_=ptT_ps, func=ACT.Sigmoid)
            # tiny same-table spacer: the scalar engine posts an instruction's
            # completion only after the NEXT queued instruction retires; this
            # keeps the 1.5us Exp table load from stalling the sig8 handoff
            spacer = sp.tile([2, 2], F32)
            nc.scalar.activation(out=spacer[:], in_=sig8[0:2, 0:2], func=ACT.Sigmoid)
            # rows 0:8 = r-part (p_t[:,0]), rows 8:16 = c-part (p_t[:,1])
            ptb_ps = ptcomb[0:16, 0:BPC]
            nc.tensor.matmul(out=ptb_ps, lhsT=selc16, rhs=sig8[:], start=True, stop=True, skip_group_check=True)

            # ---- round via the +-2^23 trick (one op) ----------------------
            prf = sp.tile([16, BPC], F32)
            nc.vector.tensor_scalar(
                out=prf[:], in0=ptb_ps, scalar1=8388608.0, scalar2=8388608.0,
                op0=AOP.add, op1=AOP.subtract,
            )

            # ---- window values: a=max(p+o,0); rr=a*(a<129); rm1=max(rr-1,0)
            aa = sp.tile([16, BPC], F32)
            nc.vector.tensor_scalar(
                out=aa[:], in0=prf[:], scalar1=oi16, scalar2=0.0,
                op0=AOP.add, op1=AOP.max,
            )
            amask = sp.tile([16, BPC], F32)
            nc.vector.tensor_scalar(
                out=amask[:], in0=aa[:], scalar1=float(ROWS), scalar2=None, op0=AOP.is_lt
            )
            rr = sp.tile([16, BPC], F32)
            nc.vector.tensor_tensor(out=rr[:], in0=aa[:], in1=amask[:], op=AOP.mult)
            # rm1z: rows 0:8 = max(r-1,0), rows 8:16 = rr_c (row 11 = p1,
            # since c_3 = clip-mod(p1) = p1 exactly)
            rm1z = sp.tile([16, BPC], F32)
            nc.vector.tensor_copy(rm1z[:], rr[:])
            nc.vector.tensor_scalar(
                out=rm1z[0:8, :], in0=rm1z[0:8, :], scalar1=1.0, scalar2=0.0,
                op0=AOP.subtract, op1=AOP.max,
            )

            # ---- block indices: one selector matmul + diag pick -----------
            # idx8[p, b'] = 128*rm1_r[i(p), b'] + p1[b']
            comb2 = pp.tile([128, BPC + 2], F32)
            idx8_ps = comb2[:, 0:BPC]
            nc.tensor.matmul(out=idx8_ps, lhsT=E12c, rhs=rm1z[:], start=True, stop=True, skip_group_check=True)
            m1 = sp.tile([128, BPC], F32)
            nc.vector.tensor_tensor(out=m1[:], in0=idx8_ps, in1=selmask, op=AOP.mult)
            red = sp.tile([128, 1], F32)
            nc.vector.tensor_reduce(out=red[:], in_=m1[:], axis=AXL.X, op=AOP.add)
            idxs = sp.tile([128, 1], F32)
            nc.vector.tensor_scalar(
                out=idxs[:], in0=red[:], scalar1=jgboff, scalar2=None, op0=AOP.add
            )
            idx128 = sp.tile([128, 1], I32)
            nc.vector.tensor_copy(idx128[:], idxs[:])

            # ---- THE gather: one DMA, 128 x 2KB blocks --------------------
            qgB = sp.tile([128, NS * D], F32R)
            nc.gpsimd.indirect_dma_start(
                out=qgB[:],
                out_offset=None,
                in_=qtab[:].bitcast(F32R),
                in_offset=IndirectOffsetOnAxis(ap=idx128[:, 0:1], axis=0),
            )

            # ---- shift/valid pre-term (overlaps the gather) ---------------
            # pre16 = 512*(rr>0) - (rm1 - p_t)^2/8 per r/c component
            rpos = sp.tile([16, BPC], F32)
            nc.vector.tensor_scalar(
                out=rpos[:], in0=rr[:], scalar1=0.0, scalar2=None, op0=AOP.is_gt
            )
            rm1f = sp.tile([16, BPC], F32)
            nc.vector.tensor_scalar(
                out=rm1f[:], in0=rr[:], scalar1=1.0, scalar2=0.0,
                op0=AOP.subtract, op1=AOP.max,
            )
            rexpd = sp.tile([16, BPC], F32)
            nc.vector.tensor_tensor(out=rexpd[:], in0=rm1f[:], in1=ptb_ps, op=AOP.subtract)
            sq = sp.tile([16, BPC], F32)
            nc.vector.tensor_tensor(out=sq[:], in0=rexpd[:], in1=rexpd[:], op=AOP.mult)
            tsA = sp.tile([16, BPC], F32)
            nc.vector.tensor_scalar(
                out=tsA[:], in0=sq[:], scalar1=-0.125, scalar2=None, op0=AOP.mult
            )
            pre16 = sp.tile([16, BPC], F32)
            nc.vector.scalar_tensor_tensor(
                out=pre16[:], in0=rpos[:], scalar=HALFC, in1=tsA[:],
                op0=AOP.mult, op1=AOP.add,
            )

            # ---- vT[d,b] = sum_c W_a[c,d] c_t[c,b] ------------------------
            vT_ps = pp.tile([D, BPC], F32)
            nc.tensor.matmul(out=vT_ps[:], lhsT=wa0, rhs=ct0, start=True, stop=False)
            nc.tensor.matmul(out=vT_ps[:], lhsT=wa1, rhs=ct1, start=False, stop=True)
            vT_sb = sp.tile([D, BPC], F32R)
            nc.vector.tensor_copy(vT_sb[:], vT_ps[:])

            # ---- scores PSUM: per-batch consts + banded shift/mask --------
            # (1024-scale constants: order-independent, no cancellation loss)
            scores_ps = pp.tile([128, NS * BPC], F32)
            nc.tensor.matmul(
                out=scores_ps[:], lhsT=bsel, rhs=constm8,
                start=True, stop=False, skip_group_check=True,
            )
            for s in range(NS):
                nc.tensor.matmul(
                    out=scores_ps[:, s * BPC : (s + 1) * BPC],
                    lhsT=aL[0:16, 384 + 128 * s : 384 + 128 * (s + 1)],
                    rhs=pre16[:],
                    start=False, stop=False, skip_group_check=True,
                )

            # ---- transpose gathered strips, score matmuls (fp32r) ---------
            qgT_sb = sp.tile([D, NS * 128], F32R)
            for s in range(NS):
                tr_ps = pp.tile([D, 128], F32, tag=f"tr{s % 2}")
                nc.tensor.transpose(tr_ps[:], qgB[:, s * D : (s + 1) * D].bitcast(F32), ident)
                nc.vector.tensor_copy(qgT_sb[:, s * 128 : (s + 1) * 128], tr_ps[:])
                nc.tensor.matmul(
                    out=scores_ps[:, s * BPC : (s + 1) * BPC],
                    lhsT=qgT_sb[:, s * 128 : (s + 1) * 128],
                    rhs=vT_sb[:],
                    start=False, stop=(s == NS - 1), skip_group_check=True,
                )

            # ---- exp straight out of PSUM ---------------------------------
            e_sb = sp.tile([128, NS * BPC], F32R)
            nc.scalar.activation(out=e_sb[:], in_=scores_ps[:], func=ACT.Exp)

            # ---- denominator: S32[8s+b'] = sum_p e[p, 8s+b'] --------------
            S32_ps = comb2[0 : NS * BPC, BPC : BPC + 2]
            nc.tensor.matmul(out=S32_ps, lhsT=e_sb[:], rhs=ones2[:], start=True, stop=True, skip_group_check=True)

            # ---- unnormalized weighted sum (fp32r) ------------------------
            out_ps = pp.tile([BPC, D], F32)
            for s in range(NS):
                nc.tensor.matmul(
                    out=out_ps[:],
                    lhsT=e_sb[:, s * BPC : (s + 1) * BPC],
                    rhs=qgB[:, s * D : (s + 1) * D],
                    start=(s == 0), stop=(s == NS - 1),
                )

            # ---- fold strip sums via a constant matmul, normalize, store --
            S32_sb = sp.tile([NS * BPC, 2], F32)
            nc.vector.tensor_copy(S32_sb[:], S32_ps)
            S8_ps = pp.tile([BPC, 2], F32)
            nc.tensor.matmul(out=S8_ps[:], lhsT=fold8, rhs=S32_sb[:], start=True, stop=True)
            sinv = sp.tile([BPC, 1], F32)
            nc.vector.reciprocal(sinv[:], S8_ps[:, 0:1])
            outf = sp.tile([BPC, D], F32)
            nc.vector.tensor_scalar(
                out=outf[:], in0=out_ps[:], scalar1=sinv[:, 0:1], scalar2=None,
                op0=AOP.mult,
            )
            nc.sync.dma_start(out=out[:], in_=outf[:])

    nc.compile()
    return nc


_CACHE = {}


def _prep_in_maps(q, c_t, W_a, W_p):
    offs = (np.arange(R) - (R // 2 - 1)).astype(np.float32)  # [-3..4]
    p = np.arange(128)
    b_of = p // 16
    i_of = (p % 16) // 2
    jg_of = p % 2

    selc16_np = np.zeros((2, 16), np.float32)
    selc16_np[0, 0:8] = float(H)
    selc16_np[1, 8:16] = float(H)

    oi16_np = offs[np.arange(16) % 8]

    E12c_np = np.zeros((16, 128), np.float32)
    E12c_np[i_of, p] = float(W)     # 128 * rm1_r[i(p), :]
    E12c_np[11, :] = 1.0            # + p1 (rr row 11 = c_3 = p1)

    selmask_np = (np.arange(BPC)[None, :] == b_of[:, None]).astype(np.float32)
    jgboff_np = (b_of * BSTR + PAD - 4 + 4 * jg_of).astype(np.float32)

    auxS_np = np.zeros((128, AUXS_W), np.float32)
    auxS_np[:, 16:18] = W_p.T.astype(np.float32)[0:128]
    auxS_np[:, 18:20] = W_p.T.astype(np.float32)[128:256]
    auxS_np[0:2, 20:36] = selc16_np
    auxS_np[0:16, 36] = oi16_np
    auxS_np[0:16, 37:165] = E12c_np
    auxS_np[:, 165:173] = selmask_np
    auxS_np[:, 173] = jgboff_np

    # B_s[q, p] = d(q, i(p)) + d(q, 8 + j(p, s)),  j = 4*jg(p) + s
    auxL_np = np.zeros((128, AUXL_W), np.float32)
    auxL_np[:, 0:128] = W_a.astype(np.float32)[0:128]
    auxL_np[:, 128:256] = W_a.astype(np.float32)[128:256]
    auxL_np[:, 256:384] = np.eye(128, dtype=np.float32)
    for s in range(NS):
        Bs = np.zeros((16, 128), np.float32)
        Bs[i_of, p] += 1.0
        Bs[8 + 4 * jg_of + s, p] += 1.0
        auxL_np[0:16, 384 + 128 * s : 384 + 128 * (s + 1)] = Bs
    bsel_np = (b_of[None, :] == np.arange(8)[:, None]).astype(np.float32)
    auxL_np[0:8, 896:1024] = bsel_np
    constm8_np = np.full((8, NS * BPC), FAKEC, np.float32)
    for r in range(8):
        for s in range(NS):
            constm8_np[r, 8 * s + r] = -BIGC
    auxL_np[0:8, 1024:1056] = constm8_np
    fold8_np = np.zeros((32, 8), np.float32)
    fold8_np[np.arange(32), np.arange(32) % 8] = 1.0
    auxL_np[0:32, 1056:1064] = fold8_np

    in_maps = []
    for c in range(NCORES):
        qs = q[c * BPC : (c + 1) * BPC]  # [BPC, D, H, W]
        qhw_np = np.ascontiguousarray(qs.transpose(0, 2, 3, 1)).reshape(BPC, HW, D)
        qtab_np = np.zeros((BPC, BSTR, D), np.float32)
        qtab_np[:, PAD : PAD + HW, :] = qhw_np
        ctT_np = np.ascontiguousarray(c_t[c * BPC : (c + 1) * BPC].T)  # [CSZ, BPC]
        auxS_c = auxS_np.copy()
        auxS_c[:, 0:8] = ctT_np[0:128]
        auxS_c[:, 8:16] = ctT_np[128:256]
        in_maps.append({
            "qtab": qtab_np.reshape(BPC * BSTR, D),
            "auxS": auxS_c,
            "auxL": auxL_np,
        })
    return in_maps


def run(trace=False, tmpdir=None, **inputs):
    q = np.asarray(inputs["q"], dtype=np.float32)
    c_t = np.asarray(inputs["c_t"], dtype=np.float32)
    W_a = np.asarray(inputs["W_a"], dtype=np.float32)
    W_p = np.asarray(inputs["W_p"], dtype=np.float32)
    if "nc" not in _CACHE:
        _CACHE["nc"] = _build()
    in_maps = _prep_in_maps(q, c_t, W_a, W_p)
    res = _bu.run_bass_kernel_spmd(
        _CACHE["nc"], in_maps, core_ids=list(range(NCORES)), trace=trace,
        tmpdir=tmpdir,
    )
    outp = np.concatenate([r["out"] for r in res.results], axis=0)
    return outp, res


def kernel(**inputs):
    outp, _ = run(trace=False, **inputs)
    return outp


# revision 23
# speedup vs baseline: 1.2128x; 1.0374x over previous
"""LocalAttention2d Bass kernel for 8 Trainium2 NeuronCores.

Strategy: pure data parallel over batch (8 batches/core).  The module only
attends over an 8x8 window of data-dependent spatial positions per batch.
All valid window columns are literally p1+offs (clipping only produces
invalid, masked positions), so each (batch, window-row, col-half) is ONE
contiguous 4-row block of the spatial table.  The kernel computes 128 block
indices on-device and fetches all 512 needed rows with a SINGLE indirect
DMA of 128 x 2KB descriptors from a host-padded [B*(H*W+16), D] table (the
+-8 row pads absorb edge blocks; those positions are invalid and masked).

Layout: partition p = b*16 + i*2 + jg (batch, window-row, col-half); the
index math runs on tiny [16, 8] r/c tiles, is expanded to [128, 1] block
indices by one selector matmul, and the Gaussian shift + validity mask are
folded into the scores PSUM with per-batch-constant and banded matmuls
using the linear mask -1024 + 512*(vr+vc) (exact in f32 at this scale), so
exp() reads PSUM directly.  Softmax is unnormalized: the weighted sum and
the denominator are both matmuls against exp scores, normalized at the end
by a per-partition reciprocal.  Score and weighted-sum matmuls run in
fp32r (single PE pass).

Host-side work is limited to data-INdependent layout prep (transposes of
q / c_t / W_p, constant tables, zero padding); every data-dependent step
(p_t, rounding, window indices, shift, softmax, output) runs on-device.
"""

import sys

import numpy as np

try:
    import concourse.bass_utils as _bu
except ImportError:  # fresh grading dir: fall back to the repo checkout
    sys.path.insert(0, "/opt/trn_rl_repo")
    import concourse.bass_utils as _bu

import concourse.bacc as bacc
import concourse.bass as bass
import concourse.mybir as mybir
import concourse.tile as tile
from concourse.bass import IndirectOffsetOnAxis

B, D, H, W = 64, 128, 128, 128
CSZ = 256
R = 8                     # window rows == cols
NCORES = 8
BPC = B // NCORES         # batches per core
HW = H * W
PADB = 132                # zero rows before each batch (absorbs rr_r=0 blocks)
PADE = 8                  # zero rows after each batch
BSTR = HW + PADB + PADE   # padded batch stride (rows)
ROWS = H + 1              # 129, padded row count in the reference
NS = 4                    # strips (col within 4-row block)
F32 = mybir.dt.float32
F32R = mybir.dt.float32r
I32 = mybir.dt.int32

AOP = mybir.AluOpType
ACT = mybir.ActivationFunctionType
AXL = mybir.AxisListType

BIGC = 1024.0             # mask constants: exact cancellation at 2^10 scale
HALFC = 512.0
FAKEC = -2048.0

# auxS [128, 174] (critical-path constants, first DMA):
#   0:8 ct0 | 8:16 ct1 | 16:18 wp0 | 18:20 wp1 | 20:36 selc16 (parts 0:2)
#   | 36:37 oi16 (parts 0:16) | 37:165 E12c (parts 0:16)
#   | 165:173 selmask | 173:174 jgboff
AUXS_W = 174
# auxL [128, 1064]: 0:128 wa0 | 128:256 wa1 | 256:384 ident
#   | 384:896 B_s x4 (parts 0:16) | 896:1024 bsel (parts 0:8)
#   | 1024:1056 constm8 (parts 0:8) | 1056:1064 fold8 (parts 0:32)
AUXL_W = 1064


def _build():
    nc = bacc.Bacc(
        "TRN2",
        target_bir_lowering=False,
        debug=False,
        num_devices=NCORES,
    )

    qtab = nc.dram_tensor("qtab", [BPC * BSTR, D], F32, kind="ExternalInput")
    auxS = nc.dram_tensor("auxS", [128, AUXS_W], F32, kind="ExternalInput")
    auxL = nc.dram_tensor("auxL", [128, AUXL_W], F32, kind="ExternalInput")
    out = nc.dram_tensor("out", [BPC, D], F32, kind="ExternalOutput")

    with tile.TileContext(nc) as tc:
        with (
            tc.tile_pool(name="sb", bufs=1) as sp,
            tc.tile_pool(name="ps", bufs=1, space="PSUM") as pp,
        ):
            # ---- input DMAs: small critical constants first ---------------
            aS = sp.tile([128, AUXS_W], F32)
            nc.sync.dma_start(out=aS[:], in_=auxS[:])
            aL = sp.tile([128, AUXL_W], F32)
            nc.sync.dma_start(out=aL[:], in_=auxL[:])

            ct0 = aS[:, 0:8]
            ct1 = aS[:, 8:16]
            wp0 = aS[:, 16:18]
            wp1 = aS[:, 18:20]
            selc16 = aS[0:2, 20:36]
            oi16 = aS[0:16, 36:37]
            E12c = aS[0:16, 37:165]
            selmask = aS[:, 165:173]
            jgboff = aS[:, 173:174]
            wa0 = aL[:, 0:128]
            wa1 = aL[:, 128:256]
            ident = aL[:, 256:384]
            bsel = aL[0:8, 896:1024]
            constm8 = aL[0:8, 1024:1056]
            fold8 = aL[0:32, 1056:1064]

            ones2 = sp.tile([128, 2], F32R)
            nc.gpsimd.memset(ones2[:].bitcast(F32), 1.0)

            # ---- p_t: ptT[2,8] = (W_p c_t)^T, spread 128*sigmoid to [16,8] -
            ptcomb = pp.tile([16, 2 * BPC], F32)
            ptT_ps = ptcomb[0:2, BPC : 2 * BPC]
            nc.tensor.matmul(out=ptT_ps, lhsT=wp0, rhs=ct0, start=True, stop=False, skip_group_check=True)
            nc.tensor.matmul(out=ptT_ps, lhsT=wp1, rhs=ct1, start=False, stop=True, skip_group_check=True)
            sig8 = sp.tile([2, BPC], F32)
            nc.scalar.activation(out=sig8[:], in_=ptT_ps, func=ACT.Sigmoid)
            # rows 0:8 = r-part (p_t[:,0]), rows 8:16 = c-part (p_t[:,1])
            ptb_ps = ptcomb[0:16, 0:BPC]
            nc.tensor.matmul(out=ptb_ps, lhsT=selc16, rhs=sig8[:], start=True, stop=True, skip_group_check=True)

            # ---- round via the +-2^23 trick (one op) ----------------------
            prf = sp.tile([16, BPC], F32)
            nc.vector.tensor_scalar(
                out=prf[:], in0=ptb_ps, scalar1=8388608.0, scalar2=8388608.0,
                op0=AOP.add, op1=AOP.subtract,
            )

            # ---- window values: a=max(p+o,0); rr=a*(a<129); rm1=max(rr-1,0)
            aa = sp.tile([16, BPC], F32)
            nc.vector.tensor_scalar(
                out=aa[:], in0=prf[:], scalar1=oi16, scalar2=0.0,
                op0=AOP.add, op1=AOP.max,
            )
            amask = sp.tile([16, BPC], F32)
            nc.vector.tensor_scalar(
                out=amask[:], in0=aa[:], scalar1=float(ROWS), scalar2=None, op0=AOP.is_lt
            )
            rr = sp.tile([16, BPC], F32)
            nc.vector.tensor_tensor(out=rr[:], in0=aa[:], in1=amask[:], op=AOP.mult)
            # ---- block indices: one selector matmul + diag pick -----------
            # idx8[p, b'] = 128*(rr_r[i(p), b'] - 1) + p1[b']; the -128 is
            # folded into jgboff and invalid rows (rr_r=0) land in the front
            # pad (row 11 of rr = c_3 = clip-mod(p1) = p1 exactly)
            comb2 = pp.tile([128, BPC + 2], F32)
            idx8_ps = comb2[:, 0:BPC]
            nc.tensor.matmul(out=idx8_ps, lhsT=E12c, rhs=rr[:], start=True, stop=True, skip_group_check=True)
            m1 = sp.tile([128, BPC], F32)
            nc.vector.tensor_tensor(out=m1[:], in0=idx8_ps, in1=selmask, op=AOP.mult)
            red = sp.tile([128, 1], F32)
            nc.vector.tensor_reduce(out=red[:], in_=m1[:], axis=AXL.X, op=AOP.add)
            idxs = sp.tile([128, 1], F32)
            nc.vector.tensor_scalar(
                out=idxs[:], in0=red[:], scalar1=jgboff, scalar2=None, op0=AOP.add
            )
            idx128 = sp.tile([128, 1], I32)
            nc.vector.tensor_copy(idx128[:], idxs[:])

            # ---- THE gather: one DMA, 128 x 2KB blocks --------------------
            qgB = sp.tile([128, NS * D], F32R)
            nc.gpsimd.indirect_dma_start(
                out=qgB[:],
                out_offset=None,
                in_=qtab[:].bitcast(F32R),
                in_offset=IndirectOffsetOnAxis(ap=idx128[:, 0:1], axis=0),
            )

            # ---- shift/valid pre-term (overlaps the gather) ---------------
            # pre16 = 512*(rr>0) - (rm1 - p_t)^2/8 per r/c component
            rpos = sp.tile([16, BPC], F32)
            nc.vector.tensor_scalar(
                out=rpos[:], in0=rr[:], scalar1=0.0, scalar2=None, op0=AOP.is_gt
            )
            rm1f = sp.tile([16, BPC], F32)
            nc.vector.tensor_scalar(
                out=rm1f[:], in0=rr[:], scalar1=1.0, scalar2=0.0,
                op0=AOP.subtract, op1=AOP.max,
            )
            rexpd = sp.tile([16, BPC], F32)
            nc.vector.tensor_tensor(out=rexpd[:], in0=rm1f[:], in1=ptb_ps, op=AOP.subtract)
            sq = sp.tile([16, BPC], F32)
            nc.vector.tensor_tensor(out=sq[:], in0=rexpd[:], in1=rexpd[:], op=AOP.mult)
            tsA = sp.tile([16, BPC], F32)
            nc.vector.tensor_scalar(
                out=tsA[:], in0=sq[:], scalar1=-0.125, scalar2=None, op0=AOP.mult
            )
            pre16 = sp.tile([16, BPC], F32)
            nc.vector.scalar_tensor_tensor(
                out=pre16[:], in0=rpos[:], scalar=HALFC, in1=tsA[:],
                op0=AOP.mult, op1=AOP.add,
            )

            # ---- vT[d,b] = sum_c W_a[c,d] c_t[c,b] ------------------------
            vT_ps = pp.tile([D, BPC], F32)
            nc.tensor.matmul(out=vT_ps[:], lhsT=wa0, rhs=ct0, start=True, stop=False)
            nc.tensor.matmul(out=vT_ps[:], lhsT=wa1, rhs=ct1, start=False, stop=True)
            vT_sb = sp.tile([D, BPC], F32R)
            nc.vector.tensor_copy(vT_sb[:], vT_ps[:])

            # ---- scores PSUM: per-batch consts + banded shift/mask --------
            # (1024-scale constants: order-independent, no cancellation loss)
            scores_ps = pp.tile([128, NS * BPC], F32)
            nc.tensor.matmul(
                out=scores_ps[:], lhsT=bsel, rhs=constm8,
                start=True, stop=False, skip_group_check=True,
            )
            for s in range(NS):
                nc.tensor.matmul(
                    out=scores_ps[:, s * BPC : (s + 1) * BPC],
                    lhsT=aL[0:16, 384 + 128 * s : 384 + 128 * (s + 1)],
                    rhs=pre16[:],
                    start=False, stop=False, skip_group_check=True,
                )

            # ---- transpose gathered strips, score matmuls (fp32r) ---------
            qgT_sb = sp.tile([D, NS * 128], F32R)
            for s in range(NS):
                tr_ps = pp.tile([D, 128], F32, tag=f"tr{s % 2}")
                nc.tensor.transpose(tr_ps[:], qgB[:, s * D : (s + 1) * D].bitcast(F32), ident)
                nc.vector.tensor_copy(qgT_sb[:, s * 128 : (s + 1) * 128], tr_ps[:])
                nc.tensor.matmul(
                    out=scores_ps[:, s * BPC : (s + 1) * BPC],
                    lhsT=qgT_sb[:, s * 128 : (s + 1) * 128],
                    rhs=vT_sb[:],
                    start=False, stop=(s == NS - 1), skip_group_check=True,
                )

            # ---- exp straight out of PSUM ---------------------------------
            e_sb = sp.tile([128, NS * BPC], F32R)
            nc.scalar.activation(out=e_sb[:], in_=scores_ps[:], func=ACT.Exp)

            # ---- denominator: S32[8s+b'] = sum_p e[p, 8s+b'] --------------
            S32_ps = comb2[0 : NS * BPC, BPC : BPC + 2]
            nc.tensor.matmul(out=S32_ps, lhsT=e_sb[:], rhs=ones2[:], start=True, stop=True, skip_group_check=True)

            # ---- unnormalized weighted sum (fp32r) ------------------------
            out_ps = pp.tile([BPC, D], F32)
            for s in range(NS):
                nc.tensor.matmul(
                    out=out_ps[:],
                    lhsT=e_sb[:, s * BPC : (s + 1) * BPC],
                    rhs=qgB[:, s * D : (s + 1) * D],
                    start=(s == 0), stop=(s == NS - 1),
                )

            # ---- fold strip sums via a constant matmul, normalize, store --
            S32_sb = sp.tile([NS * BPC, 2], F32)
            nc.vector.tensor_copy(S32_sb[:], S32_ps)
            S8_ps = pp.tile([BPC, 2], F32)
            nc.tensor.matmul(out=S8_ps[:], lhsT=fold8, rhs=S32_sb[:], start=True, stop=True)
            sinv = sp.tile([BPC, 1], F32)
            nc.vector.reciprocal(sinv[:], S8_ps[:, 0:1])
            outf = sp.tile([BPC, D], F32)
            nc.vector.tensor_scalar(
                out=outf[:], in0=out_ps[:], scalar1=sinv[:, 0:1], scalar2=None,
                op0=AOP.mult,
            )
            nc.sync.dma_start(out=out[:], in_=outf[:])

    nc.compile()
    return nc


_CACHE = {}


def _prep_in_maps(q, c_t, W_a, W_p):
    offs = (np.arange(R) - (R // 2 - 1)).astype(np.float32)  # [-3..4]
    p = np.arange(128)
    b_of = p // 16
    i_of = (p % 16) // 2
    jg_of = p % 2

    selc16_np = np.zeros((2, 16), np.float32)
    selc16_np[0, 0:8] = float(H)
    selc16_np[1, 8:16] = float(H)

    oi16_np = offs[np.arange(16) % 8]

    E12c_np = np.zeros((16, 128), np.float32)
    E12c_np[i_of, p] = float(W)     # 128 * rm1_r[i(p), :]
    E12c_np[11, :] = 1.0            # + p1 (rr row 11 = c_3 = p1)

    selmask_np = (np.arange(BPC)[None, :] == b_of[:, None]).astype(np.float32)
    jgboff_np = (b_of * BSTR + PADB - W - 4 + 4 * jg_of).astype(np.float32)

    auxS_np = np.zeros((128, AUXS_W), np.float32)
    auxS_np[:, 16:18] = W_p.T.astype(np.float32)[0:128]
    auxS_np[:, 18:20] = W_p.T.astype(np.float32)[128:256]
    auxS_np[0:2, 20:36] = selc16_np
    auxS_np[0:16, 36] = oi16_np
    auxS_np[0:16, 37:165] = E12c_np
    auxS_np[:, 165:173] = selmask_np
    auxS_np[:, 173] = jgboff_np

    # B_s[q, p] = d(q, i(p)) + d(q, 8 + j(p, s)),  j = 4*jg(p) + s
    auxL_np = np.zeros((128, AUXL_W), np.float32)
    auxL_np[:, 0:128] = W_a.astype(np.float32)[0:128]
    auxL_np[:, 128:256] = W_a.astype(np.float32)[128:256]
    auxL_np[:, 256:384] = np.eye(128, dtype=np.float32)
    for s in range(NS):
        Bs = np.zeros((16, 128), np.float32)
        Bs[i_of, p] += 1.0
        Bs[8 + 4 * jg_of + s, p] += 1.0
        auxL_np[0:16, 384 + 128 * s : 384 + 128 * (s + 1)] = Bs
    bsel_np = (b_of[None, :] == np.arange(8)[:, None]).astype(np.float32)
    auxL_np[0:8, 896:1024] = bsel_np
    constm8_np = np.full((8, NS * BPC), FAKEC, np.float32)
    for r in range(8):
        for s in range(NS):
            constm8_np[r, 8 * s + r] = -BIGC
    auxL_np[0:8, 1024:1056] = constm8_np
    fold8_np = np.zeros((32, 8), np.float32)
    fold8_np[np.arange(32), np.arange(32) % 8] = 1.0
    auxL_np[0:32, 1056:1064] = fold8_np

    in_maps = []
    for c in range(NCORES):
        qs = q[c * BPC : (c + 1) * BPC]  # [BPC, D, H, W]
        qhw_np = np.ascontiguousarray(qs.transpose(0, 2, 3, 1)).reshape(BPC, HW, D)
        qtab_np = np.zeros((BPC, BSTR, D), np.float32)
        qtab_np[:, PADB : PADB + HW, :] = qhw_np
        ctT_np = np.ascontiguousarray(c_t[c * BPC : (c + 1) * BPC].T)  # [CSZ, BPC]
        auxS_c = auxS_np.copy()
        auxS_c[:, 0:8] = ctT_np[0:128]
        auxS_c[:, 8:16] = ctT_np[128:256]
        in_maps.append({
            "qtab": qtab_np.reshape(BPC * BSTR, D),
            "auxS": auxS_c,
            "auxL": auxL_np,
        })
    return in_maps


def run(trace=False, tmpdir=None, **inputs):
    q = np.asarray(inputs["q"], dtype=np.float32)
    c_t = np.asarray(inputs["c_t"], dtype=np.float32)
    W_a = np.asarray(inputs["W_a"], dtype=np.float32)
    W_p = np.asarray(inputs["W_p"], dtype=np.float32)
    if "nc" not in _CACHE:
        _CACHE["nc"] = _build()
    in_maps = _prep_in_maps(q, c_t, W_a, W_p)
    res = _bu.run_bass_kernel_spmd(
        _CACHE["nc"], in_maps, core_ids=list(range(NCORES)), trace=trace,
        tmpdir=tmpdir,
    )
    outp = np.concatenate([r["out"] for r in res.results], axis=0)
    return outp, res


def kernel(**inputs):
    outp, _ = run(trace=False, **inputs)
    return outp
